# revision 27
# baseline (speedup 1.0000x reference)
"""Trainium2 Bass kernel for CharacterNet segment-mean + FC (segment_reduce).

Reference computation (per batch row b of 32):
  x = all_encoder_layers[layer_index][b]          # (512, 768)
  for t in 0..255: mean_t = mean(x[token_map[b,t]:token_map[b,t+1]])
  ote[b*256+t] = mean_t                           # (8192, 768) output 2
  rep = ote @ fc_w.T + fc_b                       # (8192, 768) output 1

Strategy: data-parallel over batch across 8 NeuronCores (4 rows/core).
Everything is computed in TRANSPOSED orientation so the PE never has to
transpose anything:
  stage 1:  oteT[h, t] = sum_s x[s, h] * Sel[s, t]   (x chunk stationary,
            one-hot/inv-count Sel streaming; psum lands h-on-partitions)
  stage 2:  repT[n, t] = sum_h fcwT[h, n] * oteT[h, t]  (bf16, w stationary)
Outputs are written to DRAM transposed (H, B_LOC*T); the host transposes
them back for free.  All HBM I/O is bf16.
"""

import os
import numpy as np
import ml_dtypes

import concourse.bass as bass
import concourse.bacc as bacc
import concourse.mybir as mybir
import concourse.tile as tile
from concourse.bass_utils import run_bass_kernel_spmd  # noqa: F401

N_CORES = 8
B, S, H, T = 32, 512, 768, 256
B_LOC = B // N_CORES          # 4 batch rows per core
NS = S // 128                 # 4 s-chunks per row
NJ = B_LOC * NS               # 16 (128,...) s-chunks per core
NH = H // 128                 # 6 h-chunks
TT = B_LOC * T                # 1024 segment columns per core

F32 = mybir.dt.float32
BF16 = mybir.dt.bfloat16
NPBF16 = ml_dtypes.bfloat16

OPT = {
    "sel_eng": os.environ.get("KERNEL_SEL_ENG", "vector"),
    "in_dma": "sync", "out_dma": os.environ.get("KERNEL_ODMA", "gpsimd"),
    "s1rng": os.environ.get("KERNEL_S1RNG", "1") == "1",
}

# Data-dependent stage-1 t-ranges: chunk j of 128 wp tokens only overlaps
# segments [lo_j, hi_j) (union over the 8 cores — one SPMD program).  Set
# by _host_prep before build_kernel compiles; None -> full [0, T) ranges.
_S1RNG: list | None = None


def _evict_copy(nc, idx, dst, src_):
    if idx % 2 == 0:
        nc.scalar.copy(dst, src_)
    else:
        nc.vector.tensor_copy(dst, src_)


def _evict_bias(nc, idx, dst, src_, bias_ap):
    if idx % 2 == 0:
        nc.scalar.add(dst, src_, bias_ap)
    else:
        nc.vector.tensor_scalar_add(dst, src_, bias_ap)


def build_kernel(reps: int = 1, loop: bool = False,
                 bias_mm: bool = False) -> bass.Bass:
    del bias_mm  # bias is always applied (zero bias is harmless)
    nc = bacc.Bacc("TRN2", target_bir_lowering=False, debug=False,
                   num_devices=N_CORES)

    x_d = nc.dram_tensor("x", (NJ * 128, H), BF16, kind="ExternalInput")
    # packed aux: cols 0..15 = seg, 16..31 = inv  (128, 32) f32
    aux_d = nc.dram_tensor("selaux", (128, 2 * NJ), F32, kind="ExternalInput")
    w_d = nc.dram_tensor("fcwb", (H, H), BF16, kind="ExternalInput")  # fc_w.T
    fcb_d = nc.dram_tensor("fcb", (128, NH), F32, kind="ExternalInput")
    repT_d = nc.dram_tensor("repT", (H, TT), BF16, kind="ExternalOutput")
    oteT_d = nc.dram_tensor("oteT", (H, TT), BF16, kind="ExternalOutput")

    x_v = x_d.rearrange("(j p) h -> p j h", p=128)        # (128, NJ, H)
    w_v = w_d.rearrange("(k p) n -> p k n", p=128)        # (128, NH, H)
    repT_v = repT_d.rearrange("(k p) t -> p k t", p=128)  # (128, NH, TT)
    oteT_v = oteT_d.rearrange("(k p) t -> p k t", p=128)

    with tile.TileContext(nc) as tc:
        with (
            tc.tile_pool(name="const", bufs=1) as cpool,
            tc.tile_pool(name="xp", bufs=1) as xpool,
            tc.tile_pool(name="selp", bufs=1) as selpool,
            tc.tile_pool(name="mp", bufs=1) as mpool,
            tc.tile_pool(name="wp", bufs=1) as wpool,
            tc.tile_pool(name="ob", bufs=1) as opool,
            tc.tile_pool(name="p1", bufs=5, space="PSUM") as p1pool,
            tc.tile_pool(name="p2", bufs=3, space="PSUM") as p2pool,
        ):
            idma = getattr(nc, OPT["in_dma"])
            odma = getattr(nc, OPT["out_dma"])

            # one-time constant: iota row 0..T-1 on every partition
            iota_t = cpool.tile([128, T], BF16, tag="iota")
            nc.gpsimd.iota(iota_t[:], pattern=[[1, T]], base=0,
                           channel_multiplier=0,
                           allow_small_or_imprecise_dtypes=True)

            # loop-invariant loads: weights / segment aux / bias
            aux_sb = cpool.tile([128, 2 * NJ], F32, tag="aux")
            idma.dma_start(aux_sb[:], aux_d[:])
            fcb_sb = cpool.tile([128, NH], F32, tag="fcb")
            idma.dma_start(fcb_sb[:], fcb_d[:])
            w_sb = wpool.tile([128, NH, H], BF16, tag="w")

            # PE warm-up: ~3.5us of dummy matmuls so the HAM clock gate
            # releases (1.2 -> 2.4 GHz) while the first x DMA is in flight.
            for _ in range(16):
                warm_ps = p1pool.tile([128, T], F32, tag="ps1")
                nc.tensor.matmul(warm_ps[:], iota_t[:, 0:128], iota_t[:],
                                 start=True, stop=True)

            rng = (_S1RNG if (OPT["s1rng"] and _S1RNG is not None)
                   else [(0, T)] * NJ)

            # Sel chunks (loop-invariant): (s in segment t) * 1/count, bf16
            sel_eng = getattr(nc, OPT["sel_eng"])
            sel_sb = []
            for j in range(NJ):
                lo, hi = rng[j]
                sel = selpool.tile([128, T], BF16, tag=f"s{j}")
                sel_eng.tensor_scalar(
                    sel[:, lo:hi], iota_t[:, lo:hi],
                    aux_sb[:, j:j + 1], aux_sb[:, NJ + j:NJ + j + 1],
                    op0=mybir.AluOpType.is_equal,
                    op1=mybir.AluOpType.mult)
                sel_sb.append(sel)

            # Ping-pong buffer sets (ph = 0/1) so the next rep's x loads are
            # issued before the For_i all-engine barrier and land during the
            # current rep's compute.
            x_sb = [[xpool.tile([128, NS, H], BF16, tag=f"x{p}{r}",
                                name=f"x{p}{r}")
                     for r in range(B_LOC)] for p in range(2)]
            # meanT halves: rows 0-1 -> cols 0:512, rows 2-3 -> 512:1024
            mT = [[mpool.tile([128, NH, 2 * T], BF16, tag=f"mT{p}{i}",
                              name=f"mT{p}{i}")
                   for i in range(2)] for p in range(2)]
            rsbT = [opool.tile([128, NH, TT], BF16, tag=f"rT{p}",
                               name=f"rT{p}")
                    for p in range(2)]

            def emit_load(ph):
                for r in range(B_LOC):
                    idma.dma_start(x_sb[ph][r][:],
                                   x_v[:, r * NS:(r + 1) * NS, :])

            def emit_compute(ph):
                def stage1(r):
                    # oteT chunk: psum[h, t] = sum_s x[s, h] * Sel[s, t]
                    # Chunk j only overlaps segments [lo, hi); the start=True
                    # matmul clears the whole psum bank's has_written bits, so
                    # later chunks overwrite-or-accumulate per element and the
                    # per-core ranges always cover every t (counts > 0).
                    half, col = r // 2, (r % 2) * T
                    for ht in range(NH):
                        ps = p1pool.tile([128, T], F32, tag="ps1")
                        for ks in range(NS):
                            j = r * NS + ks
                            lo, hi = rng[j]
                            nc.tensor.matmul(
                                ps[:, lo:hi],
                                x_sb[ph][r][:, ks, ht * 128:(ht + 1) * 128],
                                sel_sb[j][:, lo:hi],
                                start=(ks == 0), stop=(ks == NS - 1))
                        _evict_copy(nc, r * NH + ht,
                                    mT[ph][half][:, ht, col:col + T], ps[:])

                def stage2(th):
                    # repT chunk: psum[n, t] = sum_h fcwT[h, n] * meanT[h, t]
                    for nt in range(NH):
                        ps2 = p2pool.tile([128, 2 * T], F32, tag="ps2")
                        for kh in range(NH):
                            nc.tensor.matmul(
                                ps2[:],
                                w_sb[:, kh, nt * 128:(nt + 1) * 128],
                                mT[ph][th][:, kh, :],
                                start=(kh == 0), stop=(kh == NH - 1))
                        _evict_bias(nc, th * NH + nt,
                                    rsbT[ph][:, nt,
                                             th * 2 * T:(th + 1) * 2 * T],
                                    ps2[:], fcb_sb[:, nt:nt + 1])
                        if th == 1:
                            # full-t row per nt: 2KB DRAM lines, small tail
                            odma.dma_start(repT_v[:, nt, :],
                                           rsbT[ph][:, nt, :])

                # s1 first (frees this phase's x early for the prefetch),
                # outputs stream out in halves as soon as they are ready
                stage1(0)
                stage1(1)
                odma.dma_start(oteT_v[:, :, 0:2 * T], mT[ph][0][:])
                stage1(2)
                stage1(3)
                odma.dma_start(oteT_v[:, :, 2 * T:4 * T], mT[ph][1][:])
                stage2(0)
                stage2(1)

            unroll = next((u for u in (16, 8, 4, 2) if reps % u == 0), 1)
            if loop and reps > 1 and unroll > 1:
                # software pipeline: each body computes `unroll` reps on
                # ping-pong buffers, prefetching the next phase's x ahead of
                # the For_i all-engine barrier
                emit_load(0)
                idma.dma_start(w_sb[:], w_v[:])  # after x0: x gates stage 1
                with tc.For_i(0, reps // unroll, 1,
                              hint_engines=tuple(mybir.ALL_ENGINES)):
                    for u in range(unroll):
                        emit_load((u + 1) % 2)
                        emit_compute(u % 2)
            elif loop and reps > 1:
                idma.dma_start(w_sb[:], w_v[:])
                with tc.For_i(0, reps, 1,
                              hint_engines=tuple(mybir.ALL_ENGINES)):
                    emit_load(0)
                    emit_compute(0)
            else:
                for i in range(reps):
                    if i == 0:
                        emit_load(0)
                        idma.dma_start(w_sb[:], w_v[:])
                    if i + 1 < reps:
                        emit_load((i + 1) % 2)
                    emit_compute(i % 2)

    nc.compile()
    return nc


def _host_prep(all_encoder_layers, token_map, fc_w, fc_b, layer_index):
    """Slice the chosen layer and build per-core input maps."""
    layer = int(np.asarray(layer_index))
    x_full = np.asarray(all_encoder_layers)[layer]                # (B, S, H)
    tm = np.asarray(token_map).astype(np.int64)                   # (B, T+1)

    pos = np.arange(S)
    seg = np.empty((B, S), dtype=np.int64)
    for b in range(B):
        seg[b] = np.searchsorted(tm[b], pos, side="right") - 1
    valid = pos[None, :] < tm[:, -1:]
    seg = np.where(valid, np.clip(seg, 0, T - 1), T)              # (B, S)
    counts = (tm[:, 1:] - tm[:, :-1]).astype(np.float32)          # (B, T)
    inv = np.zeros((B, S), dtype=np.float32)
    bb = np.arange(B)[:, None]
    iv = seg < T
    inv[iv] = (np.float32(1.0) /
               counts[np.broadcast_to(bb, seg.shape)[iv], seg[iv]])
    inv = inv.astype(NPBF16).astype(np.float32)  # match device bf16 sel

    # stage-1 t-ranges per s-chunk j (union over cores; 8-aligned)
    global _S1RNG
    segc = np.where(seg < T, seg, -1).reshape(N_CORES, NJ, 128)  # per core
    rngs = []
    for j in range(NJ):
        sj = segc[:, j, :]
        vals = sj[sj >= 0]
        if vals.size == 0:
            rngs.append((0, 8))
            continue
        lo = (int(vals.min()) // 8) * 8
        hi = min(T, -(-(int(vals.max()) + 1) // 8) * 8)
        rngs.append((lo, hi))
    _S1RNG = rngs

    fcwT = np.ascontiguousarray(
        np.asarray(fc_w, dtype=np.float32).T.astype(NPBF16))      # (H, H)
    fcb = np.ascontiguousarray(
        np.asarray(fc_b, dtype=np.float32).reshape(NH, 128).T)    # (128, NH)

    x_bf = x_full.astype(NPBF16)                                  # (B, S, H)
    in_maps = []
    for c in range(N_CORES):
        bs = slice(c * B_LOC, (c + 1) * B_LOC)
        # (B_LOC, S) -> (128, NJ) with column j = b*NS + chunk
        seg_t = seg[bs].reshape(NJ, 128).T.astype(np.float32)
        inv_t = inv[bs].reshape(NJ, 128).T
        aux = np.ascontiguousarray(
            np.concatenate([seg_t, inv_t], axis=1))          # (128, 2*NJ)
        in_maps.append({
            "x": np.ascontiguousarray(x_bf[bs].reshape(NJ * 128, H)),
            "selaux": aux,
            "fcwb": fcwT,
            "fcb": fcb,
        })
    return in_maps


class CachedRunner:
    """Jit/compile/load the bass program once; later calls are pure executes."""

    def __init__(self, nc, donate: bool = True):
        import jax
        from jax.sharding import Mesh, PartitionSpec
        from jax.experimental.shard_map import shard_map
        from concourse import bass2jax

        bass2jax.install_neuronx_cc_hook()
        self.nc = nc
        in_names, out_names, out_avals = [], [], []
        pname = nc.partition_id_tensor.name if nc.partition_id_tensor else None
        for alloc in nc.m.functions[0].allocations:
            if not isinstance(alloc, mybir.MemoryLocationSet):
                continue
            name = alloc.memorylocations[0].name
            if alloc.kind == "ExternalInput":
                if name != pname:
                    in_names.append(name)
            elif alloc.kind == "ExternalOutput":
                shape = tuple(alloc.tensor_shape)
                dtype = mybir.dt.np(alloc.dtype)
                out_names.append(name)
                out_avals.append(jax.core.ShapedArray(shape, dtype))
        self.in_names = list(in_names)
        self.out_names = out_names
        self.out_avals = out_avals
        n_params = len(in_names)
        n_outs = len(out_names)
        all_in_names = list(in_names) + list(out_names)
        if pname is not None:
            all_in_names.append(pname)
        donate_idx = tuple(range(n_params, n_params + n_outs)) if donate else ()

        def _body(*args):
            operands = list(args)
            if pname is not None:
                operands.append(bass2jax.partition_id_tensor())
            outs = bass2jax._bass_exec_p.bind(
                *operands,
                out_avals=tuple(out_avals),
                in_names=tuple(all_in_names),
                out_names=tuple(out_names),
                lowering_input_output_aliases=(),
                sim_require_finite=False,
                sim_require_nnan=False,
                nc=nc,
            )
            return tuple(outs)

        devices = jax.devices()[:N_CORES]
        mesh = Mesh(np.asarray(devices), ("core",))
        in_specs = (PartitionSpec("core"),) * (n_params + n_outs)
        out_specs = (PartitionSpec("core"),) * n_outs
        self.mesh = mesh
        self.sharding = jax.sharding.NamedSharding(mesh, PartitionSpec("core"))
        self.sharded = jax.jit(
            shard_map(_body, mesh=mesh, in_specs=in_specs,
                      out_specs=out_specs, check_rep=False),
            donate_argnums=donate_idx, keep_unused=True)
        self._dev_args = None

    def __call__(self, in_maps):
        concat_in = [
            np.concatenate([np.asarray(in_maps[c][n]) for c in range(N_CORES)], 0)
            for n in self.in_names]
        concat_zeros = [
            np.zeros((N_CORES * a.shape[0], *a.shape[1:]), a.dtype)
            for a in self.out_avals]
        out = self.sharded(*concat_in, *concat_zeros)
        return out  # list of jax arrays, concatenated over cores on axis 0

    def prepare(self, in_maps):
        """device_put all arguments once (requires donate=False runner)."""
        import jax
        concat_in = [
            np.concatenate([np.asarray(in_maps[c][n]) for c in range(N_CORES)], 0)
            for n in self.in_names]
        concat_zeros = [
            np.zeros((N_CORES * a.shape[0], *a.shape[1:]), a.dtype)
            for a in self.out_avals]
        self._dev_args = [jax.device_put(a, self.sharding)
                          for a in concat_in + concat_zeros]
        jax.block_until_ready(self._dev_args)

    def run_prepared(self):
        return self.sharded(*self._dev_args)

    def to_maps(self, out):
        return [
            {n: np.asarray(out[i]).reshape(N_CORES, *self.out_avals[i].shape)[c]
             for i, n in enumerate(self.out_names)}
            for c in range(N_CORES)]


_RUNNER_CACHE: dict = {}


def get_runner(reps: int = 1, loop: bool = False, donate: bool = True,
               bias_mm: bool = False) -> CachedRunner:
    rng_key = (tuple(_S1RNG) if (OPT["s1rng"] and _S1RNG is not None)
               else None)
    key = (reps, loop, donate, rng_key)
    if key not in _RUNNER_CACHE:
        _RUNNER_CACHE[key] = CachedRunner(build_kernel(reps, loop), donate)
    return _RUNNER_CACHE[key]


def _unT(a):
    """(N_CORES*H, TT) bf16 transposed-per-core -> (B*T, H) f32."""
    return np.ascontiguousarray(
        np.asarray(a).reshape(N_CORES, H, TT).transpose(0, 2, 1)
    ).reshape(B * T, H).astype(np.float32)


def kernel(all_encoder_layers, input_mask, token_map, fc_w, fc_b, layer_index):
    in_maps = _host_prep(all_encoder_layers, token_map, fc_w, fc_b, layer_index)
    runner = get_runner(1)
    out = runner(in_maps)
    idx = {n: i for i, n in enumerate(runner.out_names)}
    rep = _unT(out[idx["repT"]])
    ote = _unT(out[idx["oteT"]])
    return rep, ote
